# revision 3
# baseline (speedup 1.0000x reference)
"""GCNConv on 8 TRN2 cores — v5: per-superblock deduped source pools.

The Q7 dma_gather costs ~9ns/row regardless of bytes, so the lever is
fewer gathered rows.  Host builds, per core, a pool tensor whose segment
s holds the distinct x-rows used by superblock s's edges (bf16).  Gather
indices are pool-relative (int16 safe: <= 2*SEGMAX < 32767), which kills
the per-chunk (Q) padding: groups are per destination block only
(~211k rows/core vs ~250k), and one gather per GPB superblocks.
"""
import math
import numpy as np
import ml_dtypes

import concourse.tile as tile
from concourse import bacc, mybir
from concourse.bass_utils import run_bass_kernel_spmd

F32 = mybir.dt.float32
BF16 = mybir.dt.bfloat16
I16 = mybir.dt.int16
AL = mybir.AluOpType
ACTF = mybir.ActivationFunctionType
D = 128
P = 128
N_CORES = 8
SBW = 4           # dest blocks per superblock
GPB = 1           # superblocks per gather window
CT = 34           # max tiles per gather call (Q7 idx scratch limit)
QN = 4
BF = ml_dtypes.bfloat16


def _sbs(n_blocks):
    return [list(range(s, min(s + SBW, n_blocks)))
            for s in range(0, n_blocks, SBW)]


def _schedule(tpg, n_blocks):
    tile0 = np.zeros(n_blocks, np.int64)
    gt = 0
    for sb in _sbs(n_blocks):
        for k in sb:
            tile0[k] = gt
            gt += int(tpg[k])
    return tile0, gt


def _build(n_src_pool, n_blocks, tpg, segmax, repeat=1):
    nsh_pad = n_blocks * P
    tile0, G = _schedule(tpg, n_blocks)
    IC = G * 8
    sbs = _sbs(n_blocks)
    n_sb = len(sbs)
    batches = [list(range(i, min(i + GPB, n_sb)))
               for i in range(0, n_sb, GPB)]

    nc = bacc.Bacc("TRN2", target_bir_lowering=False, debug=False,
                   num_swdge_queues=QN)
    pool = nc.dram_tensor("pool", [n_src_pool, D], BF16,
                          kind="ExternalInput")
    Wh = nc.dram_tensor("Wh", [D, D], BF16, kind="ExternalInput")
    Wresh = nc.dram_tensor("Wresh", [D, D], BF16, kind="ExternalInput")
    bvec = nc.dram_tensor("bvec", [1, D], BF16, kind="ExternalInput")
    bres = nc.dram_tensor("bres", [1, D], BF16, kind="ExternalInput")
    iotaf = nc.dram_tensor("iotaf", [P, P], BF16, kind="ExternalInput")
    idx = nc.dram_tensor("idx", [P, IC], I16, kind="ExternalInput")
    darr = nc.dram_tensor("darr", [P, G], F32, kind="ExternalInput")
    varr = nc.dram_tensor("varr", [P, G], F32, kind="ExternalInput")
    deg = nc.dram_tensor("deg", [1, nsh_pad], BF16, kind="ExternalInput")
    outT = nc.dram_tensor("outT", [D, nsh_pad], F32, kind="ExternalOutput")

    with tile.TileContext(nc) as tc:
        with tc.tile_pool(name="const", bufs=1) as cp:
            W_sb = cp.tile([D, D], BF16)
            nc.sync.dma_start(W_sb[:], Wh.ap())
            Wres_sb = cp.tile([D, D], BF16)
            nc.sync.dma_start(Wres_sb[:], Wresh.ap())
            b_sb = cp.tile([1, D], BF16)
            nc.sync.dma_start(b_sb[:], bvec.ap())
            bres_sb = cp.tile([1, D], BF16)
            nc.sync.dma_start(bres_sb[:], bres.ap())
            deg_sb = cp.tile([1, nsh_pad], BF16)
            nc.sync.dma_start(deg_sb[:], deg.ap())
            iota_f = cp.tile([P, P], BF16)
            nc.sync.dma_start(iota_f[:], iotaf.ap())
            idx_sb = cp.tile([P, IC], I16)
            nc.sync.dma_start(idx_sb[:], idx.ap())
            d_sb = cp.tile([P, G], F32)
            nc.sync.dma_start(d_sb[:], darr.ap())
            v_sb = cp.tile([P, G], F32)
            nc.sync.dma_start(v_sb[:], varr.ap())
            ones_row = cp.tile([1, SBW * P], BF16)
            nc.vector.memset(ones_row[:], 1.0)
            aggT = cp.tile([D, nsh_pad], BF16)

            for _rep in range(repeat):
                with (
                    tc.tile_pool(name="xg", bufs=5) as xg_pool,
                    tc.tile_pool(name="s", bufs=8) as s_pool,
                    tc.tile_pool(name="ps1", bufs=SBW, space="PSUM") as ps1,
                    tc.tile_pool(name="a", bufs=2) as a_pool,
                    tc.tile_pool(name="o", bufs=2) as o_pool,
                    tc.tile_pool(name="psA", bufs=2, space="PSUM") as psA_pool,
                    tc.tile_pool(name="psB", bufs=2, space="PSUM") as psB_pool,
                ):
                    gt = 0
                    cbase = 0
                    gcall = 0
                    for bi, bat in enumerate(batches):
                        bt0 = tile0[sbs[bat[0]][0]]
                        nt = int(sum(tpg[k] for s in bat for k in sbs[s]))
                        w0 = bat[0] * segmax
                        wlen = len(bat) * segmax
                        # chunk the gather: Q7 scratch holds the idx list,
                        # keep per-call nidx within the proven envelope
                        xgs = []
                        for c0 in range(0, nt, CT):
                            cw = min(CT, nt - c0)
                            nidx = cw * P
                            xg = xg_pool.tile([P, cw * P], BF16, tag="xg",
                                              name=f"xg{c0 // CT}")
                            nc.gpsimd.dma_gather(
                                xg[:].rearrange("p (t f) -> p t f", f=P),
                                pool.ap()[w0: w0 + wlen, :],
                                idx_sb[:, cbase: cbase + nidx // 16],
                                nidx, nidx, D,
                                single_packet=(nidx <= 1024),
                                queue_num=gcall % QN,
                            )
                            cbase += nidx // 16
                            gcall += 1
                            xgs.append(xg)
                        for s in bat:
                            sb = sbs[s]
                            nb = len(sb)
                            pss = {k: ps1.tile([P, P], F32, tag="ps",
                                               name=f"ps{j}")[:]
                                   for j, k in enumerate(sb)}
                            for k in sb:
                                ntk = int(tpg[k])
                                for t in range(ntk):
                                    S = s_pool.tile([P, P], BF16, name="S")
                                    nc.vector.tensor_scalar(
                                        S[:], iota_f[:],
                                        d_sb[:, gt:gt + 1],
                                        v_sb[:, gt:gt + 1],
                                        op0=AL.is_equal, op1=AL.mult,
                                    )
                                    tb = int(tile0[k] - bt0 + t)
                                    nc.tensor.matmul(
                                        out=pss[k],
                                        lhsT=xgs[tb // CT][
                                            :, (tb % CT) * P:
                                            (tb % CT) * P + P],
                                        rhs=S[:],
                                        start=(t == 0),
                                        stop=(t == ntk - 1),
                                    )
                                    gt += 1
                            s0 = sb[0] * P
                            w = nb * P
                            for k in sb:
                                nc.scalar.copy(aggT[:, k * P:(k + 1) * P],
                                               pss[k])
                            psA = psA_pool.tile([P, SBW * P], F32)
                            nc.tensor.matmul(out=psA[:, :w], lhsT=W_sb[:],
                                             rhs=aggT[:, s0:s0 + w],
                                             start=True, stop=False)
                            nc.tensor.matmul(out=psA[:, :w],
                                             lhsT=b_sb[:1, :],
                                             rhs=deg_sb[:1, s0:s0 + w],
                                             start=False, stop=True)
                            a_t = a_pool.tile([P, SBW * P], BF16)
                            nc.scalar.activation(a_t[:, :w], psA[:, :w],
                                                 ACTF.Relu)
                            psB = psB_pool.tile([P, SBW * P], F32)
                            nc.tensor.matmul(out=psB[:, :w],
                                             lhsT=Wres_sb[:],
                                             rhs=a_t[:, :w],
                                             start=True, stop=False)
                            nc.tensor.matmul(out=psB[:, :w],
                                             lhsT=bres_sb[:1, :],
                                             rhs=ones_row[:1, :w],
                                             start=False, stop=True)
                            o_t = o_pool.tile([P, SBW * P], F32)
                            nc.vector.tensor_tensor(o_t[:, :w],
                                                    psB[:, :w],
                                                    a_t[:, :w], op=AL.add)
                            nc.sync.dma_start(outT.ap()[:, s0:s0 + w],
                                              o_t[:, :w])

    nc.compile()
    return nc


def _prep(x, W, b, Wres, bres, edge_val, edge_row, edge_col):
    x = np.ascontiguousarray(np.asarray(x, np.float32))
    xh = np.ascontiguousarray(x.astype(BF))
    Wh = np.ascontiguousarray(np.asarray(W, np.float32).astype(BF))
    Wresh = np.ascontiguousarray(np.asarray(Wres, np.float32).astype(BF))
    bh = np.asarray(b, np.float32).astype(BF).reshape(1, D)
    bresh = np.asarray(bres, np.float32).astype(BF).reshape(1, D)
    edge_row = np.asarray(edge_row)
    edge_col = np.asarray(edge_col)
    edge_val = np.asarray(edge_val, np.float32)

    N = x.shape[0]
    nsh = math.ceil(N / N_CORES)
    n_blocks = math.ceil(nsh / P)
    nsh_pad = n_blocks * P
    sbs = _sbs(n_blocks)
    n_sb = len(sbs)

    shards = []
    counts_max = np.zeros(n_blocks, np.int64)
    segmax = 1
    for c in range(N_CORES):
        lo = c * nsh
        hi = min(N, lo + nsh)
        m = (edge_row >= lo) & (edge_row < hi)
        r = (edge_row[m] - lo).astype(np.int64)
        ci = edge_col[m].astype(np.int64)
        v = edge_val[m]
        blk = r >> 7
        counts = np.bincount(blk, minlength=n_blocks)
        np.maximum(counts_max, counts, out=counts_max)
        sb_id = blk // SBW
        segs = []
        pos = np.zeros(len(ci), np.int64)
        for s in range(n_sb):
            sm = sb_id == s
            seg = np.unique(ci[sm])
            segs.append(seg)
            segmax = max(segmax, len(seg))
            pos[sm] = np.searchsorted(seg, ci[sm])
        shards.append((r, ci, v, blk, sb_id, pos, segs))

    tpg = np.maximum((counts_max + P - 1) // P, 1)
    tile0, G = _schedule(tpg, n_blocks)
    IC = G * 8
    assert GPB * segmax < 32768, segmax
    n_pool = n_sb * segmax

    iota_f = np.tile(np.arange(P, dtype=np.float32), (P, 1)).astype(BF)

    in_maps = []
    for c in range(N_CORES):
        r, ci, v, blk, sb_id, pos, segs = shards[c]
        order = np.argsort(blk, kind="stable")
        r, ci, v, blk, sb_id, pos = (a[order]
                                     for a in (r, ci, v, blk, sb_id, pos))
        starts = np.zeros(n_blocks + 1, np.int64)
        np.cumsum(np.bincount(blk, minlength=n_blocks), out=starts[1:])
        ranks = np.arange(len(r), dtype=np.int64) - starts[blk]
        slot = (tile0[blk] + (ranks >> 7)) * P + (ranks & 127)

        # pool-relative index within the GPB-superblock gather window
        batch_first_sb = (sb_id // GPB) * GPB
        rel = (sb_id - batch_first_sb) * segmax + pos

        idx16 = np.zeros(G * P, np.int16)
        d_flat = np.zeros(G * P, np.float32)
        v_flat = np.zeros(G * P, np.float32)
        idx16[slot] = rel.astype(np.int16)
        d_flat[slot] = (r & 127).astype(np.float32)
        v_flat[slot] = v
        idx_h = np.tile(np.ascontiguousarray(idx16.reshape(IC, 16).T), (8, 1))
        d_h = np.ascontiguousarray(d_flat.reshape(G, P).T)
        v_h = np.ascontiguousarray(v_flat.reshape(G, P).T)

        poolm = np.zeros((n_pool, D), BF)
        for s, seg in enumerate(segs):
            if len(seg):
                poolm[s * segmax: s * segmax + len(seg)] = xh[seg]

        degv = np.zeros(nsh_pad, np.float32)
        degv[:nsh] += np.bincount(r, weights=v, minlength=nsh
                                  ).astype(np.float32)[:nsh]
        in_maps.append({
            "pool": poolm, "Wh": Wh, "Wresh": Wresh, "bvec": bh,
            "bres": bresh, "iotaf": iota_f, "idx": idx_h, "darr": d_h,
            "varr": v_h, "deg": degv.astype(BF).reshape(1, nsh_pad),
        })
    meta = dict(N=N, nsh=nsh, n_blocks=n_blocks, nsh_pad=nsh_pad,
                tpg=tpg, G=G, segmax=segmax, n_pool=n_pool)
    return in_maps, meta


def kernel(x, W, b, Wres, bres, edge_val, edge_row, edge_col):
    in_maps, meta = _prep(x, W, b, Wres, bres, edge_val, edge_row, edge_col)
    nc = _build(meta["n_pool"], meta["n_blocks"], meta["tpg"],
                meta["segmax"])
    res = run_bass_kernel_spmd(nc, in_maps, core_ids=list(range(N_CORES)))
    N, nsh = meta["N"], meta["nsh"]
    out = np.empty((N, D), np.float32)
    for c in range(N_CORES):
        lo = c * nsh
        hi = min(N, lo + nsh)
        out[lo:hi] = res.results[c]["outT"].T[: hi - lo]
    return out
